# revision 1
# baseline (speedup 1.0000x reference)
"""Causal self-attention (B=2, T=2048, d_model=1024, H=16) on 8 TRN2 NeuronCores.

Sharding: core c handles batch b = c//4 and head group g = c%4 (heads 4g..4g+3).
Each core computes QKV projection for its heads, causal attention, and a partial
output projection y_partial = attn_out @ Wo[g*256:(g+1)*256, :]. The host sums
the 4 partials per batch (the tensor-parallel all-reduce, done on host).

Layouts on device (per core):
  xT  [1024, 2048]  = x[b].T             (contraction dim on partitions)
  qT/kT [128, 2, 2048]                   (two heads packed per 128 partitions,
                                          head dim 64 on partitions)
  S^T tiles [128 keys, <=512 queries]    softmax over keys happens via PE:
                                         V' = [V | 1] so the PV matmul also
                                         produces per-query denominators.
"""
import sys

sys.path.insert(0, "/opt/trn_rl_repo")

import numpy as np

B, T, C = 2, 2048, 1024
NH_TOT = 16
HD = 64
NH = 4          # heads per core
CO = NH * HD    # 256 channels per core
NCORES = 8
SCALE = 1.0 / 32.0  # d_model ** -0.5

_compiled = None


def _build(nrep=1, trace_sim=False):
    import concourse.bass as bass  # noqa: F401
    import concourse.mybir as mybir
    import concourse.tile as tile
    from concourse import bacc

    F32 = mybir.dt.float32
    F32R = mybir.dt.float32r
    MULT = mybir.AluOpType.mult
    EXP = mybir.ActivationFunctionType.Exp

    nc = bacc.Bacc("TRN2", target_bir_lowering=False)

    xT = nc.declare_dram_parameter("xT", [C, T], F32, isOutput=False)
    wq = nc.declare_dram_parameter("wq", [C, CO], F32, isOutput=False)
    wk = nc.declare_dram_parameter("wk", [C, CO], F32, isOutput=False)
    wv = nc.declare_dram_parameter("wv", [C, CO], F32, isOutput=False)
    wo = nc.declare_dram_parameter("wo", [CO, C], F32, isOutput=False)
    mask = nc.declare_dram_parameter("mask", [128, 128], F32, isOutput=False)
    y = nc.declare_dram_parameter("y", [T, C], F32, isOutput=True)

    xT_t = xT.rearrange("(o p) t -> p o t", p=128)   # [128, 8, 2048]
    wq_t = wq.rearrange("(o p) m -> p o m", p=128)   # [128, 8, 256]
    wk_t = wk.rearrange("(o p) m -> p o m", p=128)
    wv_t = wv.rearrange("(o p) m -> p o m", p=128)
    wo_t = wo.rearrange("(o p) m -> p o m", p=128)   # [128, 2, 1024]

    with tile.TileContext(nc, trace_sim=trace_sim) as tc:
        with (
            nc.allow_low_precision(reason="float32r matmul pipeline"),
            tc.tile_pool(name="wpool", bufs=1) as wpool,
            tc.tile_pool(name="qkvpool", bufs=1) as qkvpool,
            tc.tile_pool(name="psa", bufs=2, space="PSUM") as psa,
            tc.tile_pool(name="psb", bufs=1, space="PSUM") as psb,
        ):
            wq_sb = wpool.tile([128, 8, CO], F32R, tag="wq")
            wk_sb = wpool.tile([128, 8, CO], F32R, tag="wk")
            wv_sb = wpool.tile([128, 8, CO], F32R, tag="wv")
            wo_sb = wpool.tile([128, 2, C], F32R, tag="wo")
            mask_sb = wpool.tile([128, 128], F32R, tag="mask")
            nc.sync.dma_start(wq_sb[:], wq_t[:].bitcast(F32R))
            nc.sync.dma_start(wk_sb[:], wk_t[:].bitcast(F32R))
            nc.sync.dma_start(wv_sb[:], wv_t[:].bitcast(F32R))
            nc.sync.dma_start(wo_sb[:], wo_t[:].bitcast(F32R))
            nc.sync.dma_start(mask_sb[:], mask[:].bitcast(F32R))

            qT_sb = qkvpool.tile([128, 2, T], F32R, tag="qT")
            kT_sb = qkvpool.tile([128, 2, T], F32R, tag="kT")
            # V' per (t-block, head): 64 cols of V then a ones column
            vp_sb = qkvpool.tile([128, 16, NH, HD + 1], F32R, tag="vp")
            nc.vector.memset(vp_sb[:, :, :, HD].bitcast(F32), 1.0)

            for _rep in range(nrep):
                # ---------------- Phase 1: QKV projection ----------------
                with tc.tile_pool(name="xpool", bufs=1) as xpool:
                    xT_sb = xpool.tile([128, 8, T], F32R, tag="xT")
                    for th in range(2):
                        for kc in range(8):
                            nc.sync.dma_start(
                                xT_sb[:, kc, th * 1024:(th + 1) * 1024],
                                xT_t[:, kc, th * 1024:(th + 1) * 1024].bitcast(F32R),
                            )

                    # qT/kT: [c_out pair on partitions, t free]
                    for t8 in range(2):
                        for w_sb, dst in ((wq_sb, qT_sb), (wk_sb, kT_sb)):
                            for m in range(2):
                                pq = psa.tile([128, 1024], F32, tag="s",
                                              name="pq")
                                for half in range(2):
                                    t0c = t8 * 1024 + half * 512
                                    for kc in range(8):
                                        nc.tensor.matmul(
                                            pq[:, half * 512:(half + 1) * 512],
                                            w_sb[:, kc, m * 128:(m + 1) * 128],
                                            xT_sb[:, kc, t0c:t0c + 512],
                                            start=(kc == 0),
                                            stop=(kc == 7),
                                        )
                                nc.vector.tensor_copy(
                                    dst[:, m, t8 * 1024:(t8 + 1) * 1024], pq[:]
                                )

                    # V in [t on partitions, head channels] layout
                    for tb in range(16):
                        pv = psa.tile([128, CO], F32, tag="o", name="pv")
                        for kc in range(8):
                            nc.tensor.matmul(
                                pv[:],
                                xT_sb[:, kc, tb * 128:(tb + 1) * 128],
                                wv_sb[:, kc, :],
                                start=(kc == 0),
                                stop=(kc == 7),
                            )
                        nc.vector.tensor_copy(
                            vp_sb[:, tb, :, 0:HD],
                            pv[:].rearrange("p (h d) -> p h d", h=NH),
                        )

                # ---------------- Phase 2: causal attention ----------------
                with (
                    tc.tile_pool(name="attnpool", bufs=1) as attnpool,
                    tc.tile_pool(name="etpool", bufs=6) as etpool,
                    tc.tile_pool(name="stagepool", bufs=2) as stagepool,
                    tc.tile_pool(name="bcastpool", bufs=2) as bcastpool,
                    tc.tile_pool(name="ypool", bufs=3) as ypool,
                ):
                    oT_sb = attnpool.tile([128, 2, T], F32R, tag="oT")
                    sums_sb = attnpool.tile([128, T], F32, tag="sums")
                    recip_sb = attnpool.tile([128, T], F32R, tag="recip")

                    # Two heads interleaved + S/exp pipelined one j-block
                    # ahead of PV; 1024-wide i-chunks amortize the 352-cycle
                    # fixed cost of each ACT exp instruction.
                    for pair in range(NH // 2):
                        heads = (2 * pair, 2 * pair + 1)
                        sstages = []
                        for h in heads:
                            sumstage = stagepool.tile([65, T], F32, tag="sumstage", bufs=2, name="sumstage")
                            sstages.append(sumstage)

                        for ic in range(2):
                            i_base = 1024 * ic
                            jb_last = 8 * ic + 7
                            pos = [
                                psa.tile([65, 1024], F32, tag="o", bufs=2, name="po")
                                for _ in heads
                            ]

                            def emit_s(h, jb):
                                po2, mo2 = h % 2, h // 2
                                i0 = max(i_base, 128 * jb)
                                n = i_base + 1024 - i0
                                k_h = kT_sb[64 * po2:64 * po2 + 64, mo2, :]
                                q_h = qT_sb[64 * po2:64 * po2 + 64, mo2, :]
                                ps_s = psa.tile([128, 1024], F32, tag="s", bufs=2, name="ps_s")
                                off = i0 - i_base
                                while off < 1024:
                                    w = min(512 - off % 512, 1024 - off)
                                    nc.tensor.matmul(
                                        ps_s[:, off:off + w],
                                        k_h[:, jb * 128:(jb + 1) * 128],
                                        q_h[:, i_base + off:i_base + off + w],
                                        start=True,
                                        stop=True,
                                    )
                                    off += w
                                et = etpool.tile([128, 1024], F32R, tag="et", name="et")
                                o0 = i0 - i_base
                                nc.scalar.activation(
                                    et[:, o0:1024], ps_s[:, o0:1024], EXP,
                                    scale=SCALE,
                                )
                                if 128 * jb >= i_base:
                                    nc.vector.tensor_tensor(
                                        et[:, o0:o0 + 128], et[:, o0:o0 + 128],
                                        mask_sb[:], MULT,
                                    )
                                return et, i0

                            def emit_pv(hi, jb, et, i0):
                                off = i0 - i_base
                                while off < 1024:
                                    w = min(512 - off % 512, 1024 - off)
                                    nc.tensor.matmul(
                                        pos[hi][:, off:off + w],
                                        vp_sb[:, jb, heads[hi], :],
                                        et[:, off:off + w],
                                        start=(jb == 0),
                                        stop=(jb == jb_last),
                                    )
                                    off += w

                            pending = [emit_s(h, 0) for h in heads]
                            for jb in range(jb_last + 1):
                                nxt = None
                                if jb < jb_last:
                                    nxt = [emit_s(h, jb + 1) for h in heads]
                                for hi in range(2):
                                    emit_pv(hi, jb, *pending[hi])
                                if nxt is not None:
                                    pending = nxt

                            for hi, h in enumerate(heads):
                                po2, mo2 = h % 2, h // 2
                                nc.vector.tensor_copy(
                                    oT_sb[64 * po2:64 * po2 + 64, mo2,
                                          i_base:i_base + 1024],
                                    pos[hi][0:64, :],
                                )
                                nc.vector.tensor_copy(
                                    sstages[hi][64:65, i_base:i_base + 1024],
                                    pos[hi][64:65, :],
                                )
                            # per-ic normalize: sums -> recip -> bcast -> mult
                            isl = slice(i_base, i_base + 1024)
                            for hi, h in enumerate(heads):
                                row = 32 * pair + hi
                                nc.sync.dma_start(
                                    sums_sb[row:row + 1, isl],
                                    sstages[hi][64:65, isl],
                                )
                            nc.vector.reciprocal(
                                recip_sb[32 * pair:32 * pair + 2, isl],
                                sums_sb[32 * pair:32 * pair + 2, isl],
                            )
                            for hi, h in enumerate(heads):
                                po2, mo2 = h % 2, h // 2
                                offp = 64 * po2
                                bc = bcastpool.tile(
                                    [128, 1024], F32R, tag="bc", bufs=4, name="bc"
                                )
                                nc.sync.dma_start(
                                    bc[offp:offp + 64, :],
                                    recip_sb[32 * pair + hi:32 * pair + hi + 1,
                                             None, isl].to_broadcast([1, 64, 1024]),
                                )
                                o_h = oT_sb[offp:offp + 64, mo2, isl]
                                nc.vector.tensor_tensor(
                                    o_h, o_h, bc[offp:offp + 64, :], MULT
                                )

                    # ---------------- Phase 3: output projection ----------------
                    # two t-blocks share one SBUF tile so each y DMA moves 1 MiB
                    for tb2 in range(8):
                        y2 = ypool.tile([128, 2, C], F32, tag="yt", name="y2")
                        for sub in range(2):
                            tb = 2 * tb2 + sub
                            for nk in range(2):
                                py = psa.tile([128, 1024], F32, tag="s",
                                              name="py")[:, 0:512]
                                for cp in range(2):
                                    nc.tensor.matmul(
                                        py[:],
                                        oT_sb[:, cp, tb * 128:(tb + 1) * 128],
                                        wo_sb[:, cp, nk * 512:(nk + 1) * 512],
                                        start=(cp == 0),
                                        stop=(cp == 1),
                                    )
                                dst = y2[:, sub, nk * 512:(nk + 1) * 512]
                                if nk == 0:
                                    nc.scalar.copy(dst, py[:])
                                else:
                                    nc.vector.tensor_copy(dst, py[:])
                        nc.sync.dma_start(
                            y[tb2 * 256:(tb2 + 1) * 256, :].rearrange(
                                "(b p) c -> p b c", p=128
                            ),
                            y2[:],
                        )

    nc.compile()
    return nc


def _get_nc():
    global _compiled
    if _compiled is None:
        _compiled = _build()
    return _compiled


class _Runner:
    """Compiled PJRT executor for the SPMD kernel, reusable across calls."""

    def __init__(self, nc):
        import jax
        import concourse.mybir as mybir
        from concourse import bass2jax
        from jax.experimental.shard_map import shard_map
        from jax.sharding import Mesh, PartitionSpec

        self.jax = jax
        self.nc = nc
        bass2jax.install_neuronx_cc_hook()

        partition_name = (
            nc.partition_id_tensor.name if nc.partition_id_tensor else None
        )
        in_names, out_names, out_avals, zero_outs = [], [], [], []
        for alloc in nc.m.functions[0].allocations:
            if not isinstance(alloc, mybir.MemoryLocationSet):
                continue
            name = alloc.memorylocations[0].name
            if alloc.kind == "ExternalInput":
                if name != partition_name:
                    in_names.append(name)
            elif alloc.kind == "ExternalOutput":
                out_names.append(name)
                shape = tuple(alloc.tensor_shape)
                dtype = mybir.dt.np(alloc.dtype)
                out_avals.append(jax.core.ShapedArray(shape, dtype))
                zero_outs.append(np.zeros(shape, dtype))
        self.in_names = in_names
        self.out_names = out_names
        self.out_avals = out_avals
        self.zero_outs = zero_outs
        all_names = tuple(in_names + out_names)

        if partition_name is not None:
            all_names = all_names + (partition_name,)

        def _body(*args):
            operands = list(args)
            if partition_name is not None:
                operands.append(bass2jax.partition_id_tensor())
            outs = bass2jax._bass_exec_p.bind(
                *operands,
                out_avals=tuple(out_avals),
                in_names=all_names,
                out_names=tuple(out_names),
                lowering_input_output_aliases=(),
                sim_require_finite=True,
                sim_require_nnan=True,
                nc=nc,
            )
            return tuple(outs)

        devices = jax.devices()[:NCORES]
        assert len(devices) == NCORES
        mesh = Mesh(np.asarray(devices), ("core",))
        self._sharding = jax.sharding.NamedSharding(mesh, PartitionSpec("core"))
        n_args = len(in_names) + len(out_names)
        self.fn = jax.jit(
            shard_map(
                _body,
                mesh=mesh,
                in_specs=(PartitionSpec("core"),) * n_args,
                out_specs=(PartitionSpec("core"),) * len(out_names),
                check_rep=False,
            ),
            keep_unused=True,
        )

    def device_args(self, in_maps):
        args = [
            np.concatenate([np.asarray(m[name]) for m in in_maps], axis=0)
            for name in self.in_names
        ]
        args += [
            np.zeros((NCORES * z.shape[0], *z.shape[1:]), z.dtype)
            for z in self.zero_outs
        ]
        return [self.jax.device_put(a, self._sharding) for a in args]

    def run_device(self, dev_args):
        return self.fn(*dev_args)

    def run(self, in_maps):
        out_arrs = self.fn(*self.device_args(in_maps))
        return [
            {
                name: np.asarray(out_arrs[i]).reshape(
                    NCORES, *self.out_avals[i].shape
                )[c]
                for i, name in enumerate(self.out_names)
            }
            for c in range(NCORES)
        ]


_runner = None


def _get_runner():
    global _runner
    if _runner is None:
        _runner = _Runner(_get_nc())
    return _runner


def make_in_maps(x, Wqkv, Wo):
    x = np.asarray(x, dtype=np.float32)
    Wqkv = np.asarray(Wqkv, dtype=np.float32)
    Wo = np.asarray(Wo, dtype=np.float32)
    mask = np.triu(np.ones((128, 128), dtype=np.float32))
    in_maps = []
    for c in range(NCORES):
        b, g = c // 4, c % 4
        in_maps.append({
            "xT": np.ascontiguousarray(x[b].T),
            "wq": np.ascontiguousarray(Wqkv[:, g * CO:(g + 1) * CO]),
            "wk": np.ascontiguousarray(Wqkv[:, C + g * CO:C + (g + 1) * CO]),
            "wv": np.ascontiguousarray(Wqkv[:, 2 * C + g * CO:2 * C + (g + 1) * CO]),
            "wo": np.ascontiguousarray(Wo[g * CO:(g + 1) * CO, :]),
            "mask": mask,
        })
    return in_maps


def gather_output(results):
    y = np.zeros((B, T, C), dtype=np.float32)
    for c in range(NCORES):
        y[c // 4] += results[c]["y"]
    return y


def kernel(x, Wqkv, Wo):
    runner = _get_runner()
    in_maps = make_in_maps(x, Wqkv, Wo)
    return gather_output(runner.run(in_maps))



# revision 10
# speedup vs baseline: 1.1608x; 1.1608x over previous
"""Causal self-attention (B=2, T=2048, d_model=1024, H=16) on 8 TRN2 NeuronCores.

Sharding: core c handles batch b = c//4 and head group g = c%4 (heads 4g..4g+3).
Each core computes QKV projection for its heads, causal attention, and a partial
output projection y_partial = attn_out @ Wo[g*256:(g+1)*256, :]. The host sums
the 4 partials per batch (the tensor-parallel all-reduce, done on host).

v2 (bf16 pipeline):
  - all PE operands bf16 (host-cast weights/x), f32 PSUM accumulation.
    FWL fast weight load + warm-clock matmuls.
  - softmax normalization batched at end of attention: denominators DVE-copied
    to 4 contiguous rows, one reciprocal_approx_fast, DMA-scatter to aligned
    partitions {0,32,64,96}, then per (pair, ic) a K=1 PE matmul broadcasts
    1/sum across 64 partitions into PSUM and one DVE mult normalizes both
    heads of the pair at once.  (The old per-pair reciprocal + 256KB broadcast
    DMA blocked the DVE FIFO ~14.5us per pair and let the PE clock re-throttle.)
  - xT prefetched one rep ahead (bufs=2), qT/kT double-buffered so next rep's
    QKV overlaps current rep's attention.

Layouts on device (per core):
  xT  [1024, 2048] bf16 = x[b].T          (contraction dim on partitions)
  qT/kT [128, 2, 2048] bf16               (two heads packed per 128 partitions,
                                           head dim 64 on partitions)
  S^T tiles [128 keys, <=512 queries] f32 PSUM; exp on ACT -> et bf16;
  V' = [V | 1] so the PV matmul also produces per-query denominators.
"""
import sys

sys.path.insert(0, "/opt/trn_rl_repo")

import numpy as np

B, T, C = 2, 2048, 1024
NH_TOT = 16
HD = 64
NH = 4          # heads per core
CO = NH * HD    # 256 channels per core
NCORES = 8
SCALE = 1.0 / 32.0  # d_model ** -0.5

_compiled = None


def _build(nrep=1, trace_sim=False):
    import concourse.bass as bass  # noqa: F401
    import concourse.mybir as mybir
    import concourse.tile as tile
    from concourse import bacc

    F32 = mybir.dt.float32
    F32R = mybir.dt.float32r
    BF16 = mybir.dt.bfloat16
    MULT = mybir.AluOpType.mult
    EXP = mybir.ActivationFunctionType.Exp

    nc = bacc.Bacc("TRN2", target_bir_lowering=False)

    xT = nc.declare_dram_parameter("xT", [C, T], BF16, isOutput=False)
    wq = nc.declare_dram_parameter("wq", [C, CO], BF16, isOutput=False)
    wk = nc.declare_dram_parameter("wk", [C, CO], BF16, isOutput=False)
    wv = nc.declare_dram_parameter("wv", [C, CO], BF16, isOutput=False)
    wo = nc.declare_dram_parameter("wo", [CO, C], BF16, isOutput=False)
    mask = nc.declare_dram_parameter("mask", [128, 128], BF16, isOutput=False)
    y = nc.declare_dram_parameter("y", [T, C], F32, isOutput=True)

    xT_t = xT.rearrange("(o p) t -> p o t", p=128)   # [128, 8, 2048]
    wq_t = wq.rearrange("(o p) m -> p o m", p=128)   # [128, 8, 256]
    wk_t = wk.rearrange("(o p) m -> p o m", p=128)
    wv_t = wv.rearrange("(o p) m -> p o m", p=128)
    wo_t = wo.rearrange("(o p) m -> p o m", p=128)   # [128, 2, 1024]

    with tile.TileContext(nc, trace_sim=trace_sim) as tc:
        with (
            nc.allow_low_precision(reason="bf16 matmul pipeline"),
            tc.tile_pool(name="wpool", bufs=1) as wpool,
            tc.tile_pool(name="qkvpool", bufs=1) as qkvpool,
            tc.tile_pool(name="xpool", bufs=2) as xpool,
            tc.tile_pool(name="etpool", bufs=6) as etpool,
            tc.tile_pool(name="ypool", bufs=2) as ypool,
            tc.tile_pool(name="psa", bufs=2, space="PSUM") as psa,
        ):
            wq_sb = wpool.tile([128, 8, CO], BF16, tag="wq")
            wk_sb = wpool.tile([128, 8, CO], BF16, tag="wk")
            wv_sb = wpool.tile([128, 8, CO], BF16, tag="wv")
            wo_sb = wpool.tile([128, 2, C], BF16, tag="wo")
            mask_sb = wpool.tile([128, 128], BF16, tag="mask")
            ones_sb = wpool.tile([128, HD], BF16, tag="ones")
            nc.sync.dma_start(wq_sb[:], wq_t[:])
            nc.sync.dma_start(wk_sb[:], wk_t[:])
            nc.sync.dma_start(wv_sb[:], wv_t[:])
            nc.sync.dma_start(wo_sb[:], wo_t[:])
            nc.sync.dma_start(mask_sb[:], mask[:])
            nc.vector.memset(ones_sb[:], 1.0)

            # V' per (t-block, head): 64 cols of V then a ones column
            vp_sb = qkvpool.tile([128, 16, NH, HD + 1], BF16, tag="vp")
            nc.vector.memset(vp_sb[:, :, :, HD], 1.0)
            oT_sb = qkvpool.tile([128, 2, T], BF16, tag="oT")
            sums_sb = qkvpool.tile([128, T], BF16, tag="sums")

            def load_x(xt):
                for th in range(2):
                    for kc in range(8):
                        nc.sync.dma_start(
                            xt[:, kc, th * 1024:(th + 1) * 1024],
                            xT_t[:, kc, th * 1024:(th + 1) * 1024],
                        )

            xt_cur = xpool.tile([128, 8, T], BF16, tag="xT", bufs=2)
            load_x(xt_cur)

            for _rep in range(nrep):
                xT_sb = xt_cur
                # ---------------- Phase 1: QKV projection ----------------
                qT_sb = qkvpool.tile([128, 2, T], BF16, tag="qT", bufs=2)
                kT_sb = qkvpool.tile([128, 2, T], BF16, tag="kT", bufs=2)

                # qT/kT: [c_out pair on partitions, t free]
                for t8 in range(2):
                    for w_sb, dst in ((wq_sb, qT_sb), (wk_sb, kT_sb)):
                        for m in range(2):
                            pq = psa.tile([128, 1024], F32, tag="s", name="pq")
                            for half in range(2):
                                t0c = t8 * 1024 + half * 512
                                for kc in range(8):
                                    nc.tensor.matmul(
                                        pq[:, half * 512:(half + 1) * 512],
                                        w_sb[:, kc, m * 128:(m + 1) * 128],
                                        xT_sb[:, kc, t0c:t0c + 512],
                                        start=(kc == 0),
                                        stop=(kc == 7),
                                    )
                            nc.vector.tensor_copy(
                                dst[:, m, t8 * 1024:(t8 + 1) * 1024], pq[:]
                            )

                # V in [t on partitions, head channels] layout
                for tb in range(16):
                    pv = psa.tile([128, 1024], F32, tag="s", name="pv")
                    for kc in range(8):
                        nc.tensor.matmul(
                            pv[:, 0:CO],
                            xT_sb[:, kc, tb * 128:(tb + 1) * 128],
                            wv_sb[:, kc, :],
                            start=(kc == 0),
                            stop=(kc == 7),
                        )
                    nc.vector.tensor_copy(
                        vp_sb[:, tb, :, 0:HD],
                        pv[:, 0:CO].rearrange("p (h d) -> p h d", h=NH),
                    )

                # prefetch next rep's xT while attention runs
                if _rep + 1 < nrep:
                    xt_cur = xpool.tile([128, 8, T], BF16, tag="xT", bufs=2)
                    load_x(xt_cur)

                # ---------------- Phase 2: causal attention ----------------
                for pair in range(NH // 2):
                    heads = (2 * pair, 2 * pair + 1)

                    for ic in range(2):
                        i_base = 1024 * ic
                        jb_last = 8 * ic + 7
                        pos = [
                            psa.tile([65, 1024], F32, tag=f"o{hi}",
                                     bufs=1, name=f"po{hi}")
                            for hi in range(2)
                        ]

                        def emit_s(h, jb):
                            po2, mo2 = h % 2, h // 2
                            i0 = max(i_base, 128 * jb)
                            k_h = kT_sb[64 * po2:64 * po2 + 64, mo2, :]
                            q_h = qT_sb[64 * po2:64 * po2 + 64, mo2, :]
                            ps_s = psa.tile([128, 1024], F32, tag="s",
                                            bufs=2, name="ps_s")
                            off = i0 - i_base
                            while off < 1024:
                                w = min(512 - off % 512, 1024 - off)
                                nc.tensor.matmul(
                                    ps_s[:, off:off + w],
                                    k_h[:, jb * 128:(jb + 1) * 128],
                                    q_h[:, i_base + off:i_base + off + w],
                                    start=True,
                                    stop=True,
                                )
                                off += w
                            et = etpool.tile([128, 1024], BF16, tag="et",
                                             name="et")
                            o0 = i0 - i_base
                            nc.scalar.activation(
                                et[:, o0:1024], ps_s[:, o0:1024], EXP,
                                scale=SCALE,
                            )
                            if 128 * jb >= i_base:
                                nc.vector.tensor_tensor(
                                    et[:, o0:o0 + 128], et[:, o0:o0 + 128],
                                    mask_sb[:], MULT,
                                )
                            return et, i0

                        def emit_pv(hi, jb, et, i0):
                            off = i0 - i_base
                            while off < 1024:
                                w = min(512 - off % 512, 1024 - off)
                                nc.tensor.matmul(
                                    pos[hi][:, off:off + w],
                                    vp_sb[:, jb, heads[hi], :],
                                    et[:, off:off + w],
                                    start=(jb == 0),
                                    stop=(jb == jb_last),
                                )
                                off += w

                        pending = [emit_s(h, 0) for h in heads]
                        for jb in range(jb_last + 1):
                            nxt = None
                            if jb < jb_last:
                                nxt = [emit_s(h, jb + 1) for h in heads]
                            for hi in range(2):
                                emit_pv(hi, jb, *pending[hi])
                            if nxt is not None:
                                pending = nxt

                        # stage unnormalized O^T and the denominators
                        isl = slice(i_base, i_base + 1024)
                        for hi, h in enumerate(heads):
                            po2 = h % 2
                            dst = oT_sb[64 * po2:64 * po2 + 64, pair, isl]
                            if po2 == 0:
                                nc.scalar.copy(dst, pos[hi][0:64, :])
                            else:
                                nc.vector.tensor_copy(dst, pos[hi][0:64, :])
                            nc.vector.tensor_copy(
                                sums_sb[32 * h:32 * h + 1, isl],
                                pos[hi][64:65, :],
                            )

                # ---------------- normalization (batched) ----------------
                # broadcast sums across 64 partitions via K=1 matmuls, then
                # full-lane reciprocal on the [128,1024] PSUM tile.
                for pair in range(NH // 2):
                    for ic in range(2):
                        isl = slice(1024 * ic, 1024 * ic + 1024)
                        bc = psa.tile([128, 1024], F32, tag="s", name="bc")
                        for hi in range(2):
                            h = 2 * pair + hi
                            for half in range(2):
                                csl = slice(1024 * ic + 512 * half,
                                            1024 * ic + 512 * half + 512)
                                nc.tensor.matmul(
                                    bc[64 * hi:64 * hi + 64,
                                       512 * half:512 * half + 512],
                                    ones_sb[32 * h:32 * h + 1, :],
                                    sums_sb[32 * h:32 * h + 1, csl],
                                    start=True,
                                    stop=True,
                                    tile_position=(32 * h, 64 * hi),
                                )
                        rb = qkvpool.tile([128, 1024], F32, tag="rb", bufs=2)
                        nc.vector.reciprocal_approx_fast(rb[:], bc[:])
                        o_sl = oT_sb[:, pair, isl]
                        nc.vector.tensor_tensor(o_sl, o_sl, rb[:], MULT)

                # ---------------- Phase 3: output projection ----------------
                # two t-blocks share one SBUF tile so each y DMA moves 1 MiB
                for tb2 in range(8):
                    y2 = ypool.tile([128, 2, C], F32, tag="yt", name="y2")
                    for sub in range(2):
                        tb = 2 * tb2 + sub
                        py = psa.tile([128, 1024], F32, tag="s", name="py")
                        for nk in range(2):
                            for cp in range(2):
                                nc.tensor.matmul(
                                    py[:, nk * 512:(nk + 1) * 512],
                                    oT_sb[:, cp, tb * 128:(tb + 1) * 128],
                                    wo_sb[:, cp, nk * 512:(nk + 1) * 512],
                                    start=(cp == 0),
                                    stop=(cp == 1),
                                )
                        dst = y2[:, sub, :]
                        if sub == 0:
                            nc.scalar.copy(dst, py[:])
                        else:
                            nc.vector.tensor_copy(dst, py[:])
                    nc.sync.dma_start(
                        y[tb2 * 256:(tb2 + 1) * 256, :].rearrange(
                            "(b p) c -> p b c", p=128
                        ),
                        y2[:],
                    )

    nc.compile()
    return nc


def _get_nc():
    global _compiled
    if _compiled is None:
        _compiled = _build()
    return _compiled


class _Runner:
    """Compiled PJRT executor for the SPMD kernel, reusable across calls."""

    def __init__(self, nc):
        import jax
        import concourse.mybir as mybir
        from concourse import bass2jax
        from jax.experimental.shard_map import shard_map
        from jax.sharding import Mesh, PartitionSpec

        self.jax = jax
        self.nc = nc
        bass2jax.install_neuronx_cc_hook()

        partition_name = (
            nc.partition_id_tensor.name if nc.partition_id_tensor else None
        )
        in_names, out_names, out_avals, zero_outs = [], [], [], []
        for alloc in nc.m.functions[0].allocations:
            if not isinstance(alloc, mybir.MemoryLocationSet):
                continue
            name = alloc.memorylocations[0].name
            if alloc.kind == "ExternalInput":
                if name != partition_name:
                    in_names.append(name)
            elif alloc.kind == "ExternalOutput":
                out_names.append(name)
                shape = tuple(alloc.tensor_shape)
                dtype = mybir.dt.np(alloc.dtype)
                out_avals.append(jax.core.ShapedArray(shape, dtype))
                zero_outs.append(np.zeros(shape, dtype))
        self.in_names = in_names
        self.out_names = out_names
        self.out_avals = out_avals
        self.zero_outs = zero_outs
        all_names = tuple(in_names + out_names)

        if partition_name is not None:
            all_names = all_names + (partition_name,)

        def _body(*args):
            operands = list(args)
            if partition_name is not None:
                operands.append(bass2jax.partition_id_tensor())
            outs = bass2jax._bass_exec_p.bind(
                *operands,
                out_avals=tuple(out_avals),
                in_names=all_names,
                out_names=tuple(out_names),
                lowering_input_output_aliases=(),
                sim_require_finite=True,
                sim_require_nnan=True,
                nc=nc,
            )
            return tuple(outs)

        devices = jax.devices()[:NCORES]
        assert len(devices) == NCORES
        mesh = Mesh(np.asarray(devices), ("core",))
        self._sharding = jax.sharding.NamedSharding(mesh, PartitionSpec("core"))
        n_args = len(in_names) + len(out_names)
        self.fn = jax.jit(
            shard_map(
                _body,
                mesh=mesh,
                in_specs=(PartitionSpec("core"),) * n_args,
                out_specs=(PartitionSpec("core"),) * len(out_names),
                check_rep=False,
            ),
            keep_unused=True,
        )

    def device_args(self, in_maps):
        args = [
            np.concatenate([np.asarray(m[name]) for m in in_maps], axis=0)
            for name in self.in_names
        ]
        args += [
            np.zeros((NCORES * z.shape[0], *z.shape[1:]), z.dtype)
            for z in self.zero_outs
        ]
        return [self.jax.device_put(a, self._sharding) for a in args]

    def run_device(self, dev_args):
        return self.fn(*dev_args)

    def run(self, in_maps):
        out_arrs = self.fn(*self.device_args(in_maps))
        return [
            {
                name: np.asarray(out_arrs[i]).reshape(
                    NCORES, *self.out_avals[i].shape
                )[c]
                for i, name in enumerate(self.out_names)
            }
            for c in range(NCORES)
        ]


_runner = None


def _get_runner():
    global _runner
    if _runner is None:
        _runner = _Runner(_get_nc())
    return _runner


def make_in_maps(x, Wqkv, Wo):
    import ml_dtypes

    bf16 = ml_dtypes.bfloat16
    x = np.asarray(x, dtype=np.float32)
    Wqkv = np.asarray(Wqkv, dtype=np.float32)
    Wo = np.asarray(Wo, dtype=np.float32)
    mask = np.triu(np.ones((128, 128), dtype=np.float32)).astype(bf16)
    in_maps = []
    for c in range(NCORES):
        b, g = c // 4, c % 4
        in_maps.append({
            "xT": np.ascontiguousarray(x[b].T).astype(bf16),
            "wq": np.ascontiguousarray(
                Wqkv[:, g * CO:(g + 1) * CO]).astype(bf16),
            "wk": np.ascontiguousarray(
                Wqkv[:, C + g * CO:C + (g + 1) * CO]).astype(bf16),
            "wv": np.ascontiguousarray(
                Wqkv[:, 2 * C + g * CO:2 * C + (g + 1) * CO]).astype(bf16),
            "wo": np.ascontiguousarray(Wo[g * CO:(g + 1) * CO, :]).astype(bf16),
            "mask": mask,
        })
    return in_maps


def gather_output(results):
    y = np.zeros((B, T, C), dtype=np.float32)
    for c in range(NCORES):
        y[c // 4] += results[c]["y"]
    return y


def kernel(x, Wqkv, Wo):
    runner = _get_runner()
    in_maps = make_in_maps(x, Wqkv, Wo)
    return gather_output(runner.run(in_maps))


# revision 13
# speedup vs baseline: 1.5435x; 1.3297x over previous
"""Causal self-attention (B=2, T=2048, d_model=1024, H=16) on 8 TRN2 NeuronCores.

Sharding: core c handles batch b = c//4 and head group g = c%4 (heads 4g..4g+3).
Each core computes QKV projection for its heads, causal attention, and a partial
output projection y_partial = attn_out @ Wo[g*256:(g+1)*256, :]. The host sums
the 4 partials per batch (the tensor-parallel all-reduce, done on host).

v2 (bf16 pipeline):
  - all PE operands bf16 (host-cast weights/x), f32 PSUM accumulation.
    FWL fast weight load + warm-clock matmuls.
  - softmax normalization batched at end of attention: denominators DVE-copied
    to 4 contiguous rows, one reciprocal_approx_fast, DMA-scatter to aligned
    partitions {0,32,64,96}, then per (pair, ic) a K=1 PE matmul broadcasts
    1/sum across 64 partitions into PSUM and one DVE mult normalizes both
    heads of the pair at once.  (The old per-pair reciprocal + 256KB broadcast
    DMA blocked the DVE FIFO ~14.5us per pair and let the PE clock re-throttle.)
  - xT prefetched one rep ahead (bufs=2), qT/kT double-buffered so next rep's
    QKV overlaps current rep's attention.

Layouts on device (per core):
  xT  [1024, 2048] bf16 = x[b].T          (contraction dim on partitions)
  qT/kT [128, 2, 2048] bf16               (two heads packed per 128 partitions,
                                           head dim 64 on partitions)
  S^T tiles [128 keys, <=512 queries] f32 PSUM; exp on ACT -> et bf16;
  V' = [V | 1] so the PV matmul also produces per-query denominators.
"""
import sys

sys.path.insert(0, "/opt/trn_rl_repo")

import numpy as np

B, T, C = 2, 2048, 1024
NH_TOT = 16
HD = 64
NH = 4          # heads per core
CO = NH * HD    # 256 channels per core
NCORES = 8
SCALE = 1.0 / 32.0  # d_model ** -0.5

_compiled = None


def _build(nrep=1, trace_sim=False):
    import concourse.bass as bass  # noqa: F401
    import concourse.mybir as mybir
    import concourse.tile as tile
    from concourse import bacc

    F32 = mybir.dt.float32
    F32R = mybir.dt.float32r
    BF16 = mybir.dt.bfloat16
    MULT = mybir.AluOpType.mult
    EXP = mybir.ActivationFunctionType.Exp

    nc = bacc.Bacc("TRN2", target_bir_lowering=False)

    xT = nc.declare_dram_parameter("xT", [C, T], BF16, isOutput=False)
    wq = nc.declare_dram_parameter("wq", [C, CO], BF16, isOutput=False)
    wk = nc.declare_dram_parameter("wk", [C, CO], BF16, isOutput=False)
    wv = nc.declare_dram_parameter("wv", [C, CO], BF16, isOutput=False)
    wo = nc.declare_dram_parameter("wo", [CO, C], BF16, isOutput=False)
    mask = nc.declare_dram_parameter("mask", [128, 128], BF16, isOutput=False)
    y = nc.declare_dram_parameter("y", [T, C], F32, isOutput=True)

    xT_t = xT.rearrange("(o p) t -> p o t", p=128)   # [128, 8, 2048]
    wq_t = wq.rearrange("(o p) m -> p o m", p=128)   # [128, 8, 256]
    wk_t = wk.rearrange("(o p) m -> p o m", p=128)
    wv_t = wv.rearrange("(o p) m -> p o m", p=128)
    wo_t = wo.rearrange("(o p) m -> p o m", p=128)   # [128, 2, 1024]

    with tile.TileContext(nc, trace_sim=trace_sim) as tc:
        with (
            nc.allow_low_precision(reason="bf16 matmul pipeline"),
            tc.tile_pool(name="wpool", bufs=1) as wpool,
            tc.tile_pool(name="qkvpool", bufs=1) as qkvpool,
            tc.tile_pool(name="xpool", bufs=2) as xpool,
            tc.tile_pool(name="etpool", bufs=6) as etpool,
            tc.tile_pool(name="ypool", bufs=2) as ypool,
            tc.tile_pool(name="psa", bufs=2, space="PSUM") as psa,
        ):
            wq_sb = wpool.tile([128, 8, CO], BF16, tag="wq")
            wk_sb = wpool.tile([128, 8, CO], BF16, tag="wk")
            wv_sb = wpool.tile([128, 8, CO], BF16, tag="wv")
            wo_sb = wpool.tile([128, 2, C], BF16, tag="wo")
            mask_sb = wpool.tile([128, 128], BF16, tag="mask")
            ones_sb = wpool.tile([128, HD], BF16, tag="ones")
            nc.sync.dma_start(wq_sb[:], wq_t[:])
            nc.sync.dma_start(wk_sb[:], wk_t[:])
            nc.sync.dma_start(wv_sb[:], wv_t[:])
            nc.sync.dma_start(wo_sb[:], wo_t[:])
            nc.sync.dma_start(mask_sb[:], mask[:])
            nc.vector.memset(ones_sb[:], 1.0)

            # V' ones columns (col HD of each head) are set once per slot
            # below, right after the first allocation of each vp buffer.
            oT_sb = qkvpool.tile([128, 2, T], BF16, tag="oT")
            sums_sb = qkvpool.tile([128, T], BF16, tag="sums")

            def load_x(xt):
                for th in range(2):
                    for kc in range(8):
                        nc.sync.dma_start(
                            xt[:, kc, th * 1024:(th + 1) * 1024],
                            xT_t[:, kc, th * 1024:(th + 1) * 1024],
                        )

            def qkv_phase(xT_sb):
                """QKV projection; returns (qT, kT, vp) tiles for the rep."""
                qT_sb = qkvpool.tile([128, 2, T], BF16, tag="qT", bufs=2)
                kT_sb = qkvpool.tile([128, 2, T], BF16, tag="kT", bufs=2)
                vp_sb = qkvpool.tile([128, 16, NH, HD + 1], BF16, tag="vp",
                                     bufs=2)
                nc.vector.memset(vp_sb[:, :, :, HD], 1.0)

                # qT/kT: [c_out pair on partitions, t free]
                for t8 in range(2):
                    for w_sb, dst in ((wq_sb, qT_sb), (wk_sb, kT_sb)):
                        for m in range(2):
                            pq = psa.tile([128, 1024], F32, tag="s", name="pq")
                            for half in range(2):
                                t0c = t8 * 1024 + half * 512
                                for kc in range(8):
                                    nc.tensor.matmul(
                                        pq[:, half * 512:(half + 1) * 512],
                                        w_sb[:, kc, m * 128:(m + 1) * 128],
                                        xT_sb[:, kc, t0c:t0c + 512],
                                        start=(kc == 0),
                                        stop=(kc == 7),
                                    )
                            nc.vector.tensor_copy(
                                dst[:, m, t8 * 1024:(t8 + 1) * 1024], pq[:]
                            )

                # V in [t on partitions, head channels] layout
                for tb in range(16):
                    pv = psa.tile([128, 1024], F32, tag="s", name="pv")
                    for kc in range(8):
                        nc.tensor.matmul(
                            pv[:, 0:CO],
                            xT_sb[:, kc, tb * 128:(tb + 1) * 128],
                            wv_sb[:, kc, :],
                            start=(kc == 0),
                            stop=(kc == 7),
                        )
                    nc.vector.tensor_copy(
                        vp_sb[:, tb, :, 0:HD],
                        pv[:, 0:CO].rearrange("p (h d) -> p h d", h=NH),
                    )
                return qT_sb, kT_sb, vp_sb

            def att_phase(qT_sb, kT_sb, vp_sb):
                for pair in range(NH // 2):
                    heads = (2 * pair, 2 * pair + 1)

                    for ic in range(2):
                        i_base = 1024 * ic
                        jb_last = 8 * ic + 7
                        pos = [
                            psa.tile([65, 1024], F32, tag=f"o{hi}",
                                     bufs=1, name=f"po{hi}")
                            for hi in range(2)
                        ]

                        def emit_s(h, jb):
                            po2, mo2 = h % 2, h // 2
                            i0 = max(i_base, 128 * jb)
                            k_h = kT_sb[64 * po2:64 * po2 + 64, mo2, :]
                            q_h = qT_sb[64 * po2:64 * po2 + 64, mo2, :]
                            ps_s = psa.tile([128, 1024], F32, tag="s",
                                            bufs=2, name="ps_s")
                            off = i0 - i_base
                            while off < 1024:
                                w = min(512 - off % 512, 1024 - off)
                                nc.tensor.matmul(
                                    ps_s[:, off:off + w],
                                    k_h[:, jb * 128:(jb + 1) * 128],
                                    q_h[:, i_base + off:i_base + off + w],
                                    start=True,
                                    stop=True,
                                )
                                off += w
                            et = etpool.tile([128, 1024], BF16, tag="et",
                                             name="et")
                            o0 = i0 - i_base
                            nc.scalar.activation(
                                et[:, o0:1024], ps_s[:, o0:1024], EXP,
                                scale=SCALE,
                            )
                            if 128 * jb >= i_base:
                                nc.vector.tensor_tensor(
                                    et[:, o0:o0 + 128], et[:, o0:o0 + 128],
                                    mask_sb[:], MULT,
                                )
                            return et, i0

                        def emit_pv(hi, jb, et, i0):
                            off = i0 - i_base
                            while off < 1024:
                                w = min(512 - off % 512, 1024 - off)
                                nc.tensor.matmul(
                                    pos[hi][:, off:off + w],
                                    vp_sb[:, jb, heads[hi], :],
                                    et[:, off:off + w],
                                    start=(jb == 0),
                                    stop=(jb == jb_last),
                                )
                                off += w

                        pending = [emit_s(h, 0) for h in heads]
                        for jb in range(jb_last + 1):
                            nxt = None
                            if jb < jb_last:
                                nxt = [emit_s(h, jb + 1) for h in heads]
                            for hi in range(2):
                                emit_pv(hi, jb, *pending[hi])
                            if nxt is not None:
                                pending = nxt

                        # stage unnormalized O^T and the denominators
                        isl = slice(i_base, i_base + 1024)
                        for hi, h in enumerate(heads):
                            po2 = h % 2
                            dst = oT_sb[64 * po2:64 * po2 + 64, pair, isl]
                            if po2 == 0:
                                nc.scalar.copy(dst, pos[hi][0:64, :])
                            else:
                                nc.vector.tensor_copy(dst, pos[hi][0:64, :])
                            nc.vector.tensor_copy(
                                sums_sb[32 * h:32 * h + 1, isl],
                                pos[hi][64:65, :],
                            )

            def norm_phase():
                # broadcast sums across 64 partitions via K=1 matmuls, then
                # full-lane reciprocal on the [128,1024] PSUM tile.
                for pair in range(NH // 2):
                    for ic in range(2):
                        isl = slice(1024 * ic, 1024 * ic + 1024)
                        bc = psa.tile([128, 1024], F32, tag="s", name="bc")
                        for hi in range(2):
                            h = 2 * pair + hi
                            for half in range(2):
                                csl = slice(1024 * ic + 512 * half,
                                            1024 * ic + 512 * half + 512)
                                nc.tensor.matmul(
                                    bc[64 * hi:64 * hi + 64,
                                       512 * half:512 * half + 512],
                                    ones_sb[32 * h:32 * h + 1, :],
                                    sums_sb[32 * h:32 * h + 1, csl],
                                    start=True,
                                    stop=True,
                                    tile_position=(32 * h, 64 * hi),
                                )
                        rb = qkvpool.tile([128, 1024], F32, tag="rb", bufs=2)
                        nc.vector.reciprocal_approx_fast(rb[:], bc[:])
                        o_sl = oT_sb[:, pair, isl]
                        nc.vector.tensor_tensor(o_sl, o_sl, rb[:], MULT)

            def proj_phase():
                # two t-blocks share one SBUF tile so each y DMA moves 1 MiB
                for tb2 in range(8):
                    y2 = ypool.tile([128, 2, C], F32, tag="yt", name="y2")
                    for sub in range(2):
                        tb = 2 * tb2 + sub
                        py = psa.tile([128, 1024], F32, tag="s", name="py")
                        for nk in range(2):
                            for cp in range(2):
                                nc.tensor.matmul(
                                    py[:, nk * 512:(nk + 1) * 512],
                                    oT_sb[:, cp, tb * 128:(tb + 1) * 128],
                                    wo_sb[:, cp, nk * 512:(nk + 1) * 512],
                                    start=(cp == 0),
                                    stop=(cp == 1),
                                )
                        dst = y2[:, sub, :]
                        if sub == 0:
                            nc.scalar.copy(dst, py[:])
                        else:
                            nc.vector.tensor_copy(dst, py[:])
                    nc.gpsimd.dma_start(
                        y[tb2 * 256:(tb2 + 1) * 256, :].rearrange(
                            "(b p) c -> p b c", p=128
                        ),
                        y2[:],
                    )

            # ---- software-pipelined rep loop ----
            # emission order per rep: [xT(r+1) load] att(r) [qkv(r+1)]
            # norm(r) proj(r) -- next rep's QKV fills the PE during this
            # rep's normalize/proj serialization, and xT transfers during
            # attention.
            xt_cur = xpool.tile([128, 8, T], BF16, tag="xT", bufs=2)
            load_x(xt_cur)
            cur = qkv_phase(xt_cur)
            for _rep in range(nrep):
                if _rep + 1 < nrep:
                    xt_nxt = xpool.tile([128, 8, T], BF16, tag="xT", bufs=2)
                    load_x(xt_nxt)
                att_phase(*cur)
                if _rep + 1 < nrep:
                    cur = qkv_phase(xt_nxt)
                norm_phase()
                proj_phase()

    nc.compile()
    return nc


def _get_nc():
    global _compiled
    if _compiled is None:
        _compiled = _build()
    return _compiled


class _Runner:
    """Compiled PJRT executor for the SPMD kernel, reusable across calls."""

    def __init__(self, nc):
        import jax
        import concourse.mybir as mybir
        from concourse import bass2jax
        from jax.experimental.shard_map import shard_map
        from jax.sharding import Mesh, PartitionSpec

        self.jax = jax
        self.nc = nc
        bass2jax.install_neuronx_cc_hook()

        partition_name = (
            nc.partition_id_tensor.name if nc.partition_id_tensor else None
        )
        in_names, out_names, out_avals, zero_outs = [], [], [], []
        for alloc in nc.m.functions[0].allocations:
            if not isinstance(alloc, mybir.MemoryLocationSet):
                continue
            name = alloc.memorylocations[0].name
            if alloc.kind == "ExternalInput":
                if name != partition_name:
                    in_names.append(name)
            elif alloc.kind == "ExternalOutput":
                out_names.append(name)
                shape = tuple(alloc.tensor_shape)
                dtype = mybir.dt.np(alloc.dtype)
                out_avals.append(jax.core.ShapedArray(shape, dtype))
                zero_outs.append(np.zeros(shape, dtype))
        self.in_names = in_names
        self.out_names = out_names
        self.out_avals = out_avals
        self.zero_outs = zero_outs
        all_names = tuple(in_names + out_names)

        if partition_name is not None:
            all_names = all_names + (partition_name,)

        def _body(*args):
            operands = list(args)
            if partition_name is not None:
                operands.append(bass2jax.partition_id_tensor())
            outs = bass2jax._bass_exec_p.bind(
                *operands,
                out_avals=tuple(out_avals),
                in_names=all_names,
                out_names=tuple(out_names),
                lowering_input_output_aliases=(),
                sim_require_finite=True,
                sim_require_nnan=True,
                nc=nc,
            )
            return tuple(outs)

        devices = jax.devices()[:NCORES]
        assert len(devices) == NCORES
        mesh = Mesh(np.asarray(devices), ("core",))
        self._sharding = jax.sharding.NamedSharding(mesh, PartitionSpec("core"))
        n_args = len(in_names) + len(out_names)
        self.fn = jax.jit(
            shard_map(
                _body,
                mesh=mesh,
                in_specs=(PartitionSpec("core"),) * n_args,
                out_specs=(PartitionSpec("core"),) * len(out_names),
                check_rep=False,
            ),
            keep_unused=True,
        )

    def device_args(self, in_maps):
        args = [
            np.concatenate([np.asarray(m[name]) for m in in_maps], axis=0)
            for name in self.in_names
        ]
        args += [
            np.zeros((NCORES * z.shape[0], *z.shape[1:]), z.dtype)
            for z in self.zero_outs
        ]
        return [self.jax.device_put(a, self._sharding) for a in args]

    def run_device(self, dev_args):
        return self.fn(*dev_args)

    def run(self, in_maps):
        out_arrs = self.fn(*self.device_args(in_maps))
        return [
            {
                name: np.asarray(out_arrs[i]).reshape(
                    NCORES, *self.out_avals[i].shape
                )[c]
                for i, name in enumerate(self.out_names)
            }
            for c in range(NCORES)
        ]


_runner = None


def _get_runner():
    global _runner
    if _runner is None:
        _runner = _Runner(_get_nc())
    return _runner


def make_in_maps(x, Wqkv, Wo):
    import ml_dtypes

    bf16 = ml_dtypes.bfloat16
    x = np.asarray(x, dtype=np.float32)
    Wqkv = np.asarray(Wqkv, dtype=np.float32)
    Wo = np.asarray(Wo, dtype=np.float32)
    mask = np.triu(np.ones((128, 128), dtype=np.float32)).astype(bf16)
    in_maps = []
    for c in range(NCORES):
        b, g = c // 4, c % 4
        in_maps.append({
            "xT": np.ascontiguousarray(x[b].T).astype(bf16),
            "wq": np.ascontiguousarray(
                Wqkv[:, g * CO:(g + 1) * CO]).astype(bf16),
            "wk": np.ascontiguousarray(
                Wqkv[:, C + g * CO:C + (g + 1) * CO]).astype(bf16),
            "wv": np.ascontiguousarray(
                Wqkv[:, 2 * C + g * CO:2 * C + (g + 1) * CO]).astype(bf16),
            "wo": np.ascontiguousarray(Wo[g * CO:(g + 1) * CO, :]).astype(bf16),
            "mask": mask,
        })
    return in_maps


def gather_output(results):
    y = np.zeros((B, T, C), dtype=np.float32)
    for c in range(NCORES):
        y[c // 4] += results[c]["y"]
    return y


def kernel(x, Wqkv, Wo):
    runner = _get_runner()
    in_maps = make_in_maps(x, Wqkv, Wo)
    return gather_output(runner.run(in_maps))
